# revision 39
# baseline (speedup 1.0000x reference)
"""Trainium2 Bass kernel for nn_MultiHeadAttention_46213848104966.

B=4, S=2048, D=1024, H=16, DK=10, DV=12.
Sharding: 8 cores = 4 batches x 2 q-row halves; each core computes complete
output rows Y[b, half] over all 16 heads; the host concatenates.

The ScalarE exp stream (256 x [128,1024] tiles ~= 266us) is the hard floor;
everything else is arranged to hide underneath it:
  - host passes Q/K/V pre-transposed ([D, S] fp16) + packed fp16/bf16
    weights, so there are no on-device transposes or stage copies.
  - q/k projections (fp32 psum, fp16 in) bounce through DRAM to scatter
    into 32-partition-aligned per-head slots (3 m-group DMAs per tensor
    block); v projects into vex [t, h, 13] bf16 with a ones column.
  - scoresT = kT_h^T q_h (fp16 operands, fp32 psum), exp -> ex bf16; the
    PV matmul uses ex as the STATIONARY operand (weight load is free in
    the PE) streaming vex's 13 columns; pva[s, h, 13] accumulates over
    t, col 12 = Z.
  - PSUM phasing: scores ring (8KB) + K1-3/V proj psum (5.5KB) coexist;
    pva (8KB) opens once projections drain, so attention starts ~12us in
    while setup finishes; PV for tch 0-2 is emitted as a backlog after
    setup (ex ring is deep enough for Act to run ahead).
  - epilogue per s-chunk: 1/Z (DVE), fused normalize-mul -> an bf16,
    PE-transpose, WO matmul, Y out.
"""

import numpy as np
from contextlib import ExitStack

S = 2048
SH = 1024  # q rows per core
D = 1024
H = 16
DK = 10
DV = 12
B = 4

_NC_CACHE = {}


def _build_program():
    import concourse.bass as bass
    import concourse.tile as tile
    from concourse import bacc, mybir

    f32 = mybir.dt.float32
    f16 = mybir.dt.float16
    bf16 = mybir.dt.bfloat16
    AF = mybir.ActivationFunctionType

    ndc = D // 128            # 8 d-chunks
    ntc = S // 128            # 16 t-chunks
    nsc = SH // 128           # 8 s-chunks
    NDEFER = 3                # t-chunks whose PV is emitted after setup

    nc = bacc.Bacc("TRN2", target_bir_lowering=False, debug=False, num_devices=8)

    QTd = nc.dram_tensor("QT", [D, SH], f16, kind="ExternalInput").ap()
    KTd = nc.dram_tensor("KT", [D, S], f16, kind="ExternalInput").ap()
    VTd = nc.dram_tensor("VT", [D, S], f16, kind="ExternalInput").ap()
    WALLd = nc.dram_tensor("WALL", [D, 512], f16, kind="ExternalInput").ap()
    WOAd = nc.dram_tensor("WOA", [128, D], bf16, kind="ExternalInput").ap()
    WOBd = nc.dram_tensor("WOB", [64, D], bf16, kind="ExternalInput").ap()
    IDd = nc.dram_tensor("IDN", [128, 128], bf16, kind="ExternalInput").ap()
    Yd = nc.dram_tensor("Y", [SH, D], f32, kind="ExternalOutput").ap()

    scale = float(np.float32(1.0) / np.sqrt(np.float32(10.0)))

    with tile.TileContext(nc) as tc, ExitStack() as ctx:
        consts = ctx.enter_context(tc.tile_pool(name="consts", bufs=1))
        qkvp = ctx.enter_context(tc.tile_pool(name="qkv", bufs=1))
        s80p = ctx.enter_context(tc.tile_pool(name="s80", bufs=2))
        exp_ = ctx.enter_context(tc.tile_pool(name="ex", bufs=8))
        anp = ctx.enter_context(tc.tile_pool(name="an", bufs=4))
        astp = ctx.enter_context(tc.tile_pool(name="ast", bufs=8))
        rzp = ctx.enter_context(tc.tile_pool(name="rz", bufs=2))
        ytp = ctx.enter_context(tc.tile_pool(name="yt", bufs=3))
        dramp = ctx.enter_context(tc.tile_pool(name="od", bufs=1, space="DRAM"))

        idn = consts.tile([128, 128], bf16, tag="idn")
        nc.sync.dma_start(out=idn[:], in_=IDd)
        wall = consts.tile([128, ndc, 512], f16, tag="wall")
        WALLr = WALLd.rearrange("(c p) m -> p c m", p=128)
        nc.sync.dma_start(out=wall[:, :, 0:160], in_=WALLr[:, :, 0:160])
        # (wk loads just before ktl0; wv + vtl0 are marker-chained after
        # the K0 scatter so the lead transfer FIFO stays clear)
        wosA = consts.tile([128, D], bf16, tag="wosA")
        nc.gpsimd.dma_start(out=wosA[:], in_=WOAd)
        wosB = consts.tile([64, D], bf16, tag="wosB")
        nc.gpsimd.dma_start(out=wosB[:], in_=WOBd)

        # per-dc slabs of the host-transposed tensors
        qtl = consts.tile([128, ndc, SH], f16, tag="qtl")
        ktl = consts.tile([128, ndc, S], f16, tag="ktl")
        vtl = consts.tile([128, ndc, S], f16, tag="vtl")
        # lead-path loads first (one DMA each): Q fully, K block-0 cols,
        # V t-chunks 0-3 cols; the bulk arrives while attention runs
        QTr = QTd.rearrange("(c p) s -> p c s", p=128)
        KTr = KTd.rearrange("(c p) s -> p c s", p=128)
        VTr = VTd.rearrange("(c p) s -> p c s", p=128)
        nc.sync.dma_start(out=qtl[:, :, 0:512], in_=QTr[:, :, 0:512])
        nc.sync.dma_start(out=qtl[:, :, 512:1024], in_=QTr[:, :, 512:1024])
        nc.sync.dma_start(out=ktl[:, :, 0:512], in_=KTr[:, :, 0:512])
        nc.sync.dma_start(out=wall[:, :, 160:512], in_=WALLr[:, :, 160:512])
        nc.sync.dma_start(out=vtl[:, :, 0:512], in_=VTr[:, :, 0:512])
        # bulk pieces carry a late scheduler priority so every lead-path
        # DMA beats them into the transfer FIFO; data deps still pull each
        # piece in before its first consumer

        # head h -> partitions 32*(h%3)..+10 of chunk h//3
        kT = qkvp.tile([128, 6, S], f16, tag="kT")
        qT = qkvp.tile([128, 6, SH], f16, tag="qT")
        vex = qkvp.tile([128, ntc, H, DV + 1], bf16, tag="vex")
        nc.vector.memset(vex[:, :, :, DV], 1.0)

        qTdr = dramp.tile([H * DK, SH], f16, tag="qTdr")
        kTdr = dramp.tile([H * DK, S], f16, tag="kTdr")

        def scatter(dma_eng, td, tgt, c0, c1):
            # src rows 30c'+10m+k -> dest partitions 32m+k, chunk c'
            w = c1 - c0
            for m in range(3):
                nch = 6 if m == 0 else 5
                tda = td[:]
                src = bass.AP(
                    tensor=tda.tensor,
                    offset=tda.offset + (10 * m) * tda.ap[0][0] + c0,
                    ap=[[tda.ap[0][0], DK], [30 * tda.ap[0][0], nch], [1, w]],
                )
                dma_eng.dma_start(
                    out=tgt[32 * m:32 * m + DK, 0:nch, c0:c1], in_=src)

        # ---------------- psum pools ----------------
        # ps ring (8KB) + pva (8KB) fill PSUM; K1-3/V projection psum
        # borrows ps-ring slots (tag "ps") so everything coexists.
        psp = ctx.enter_context(tc.tile_pool(name="ps", bufs=2, space="PSUM"))
        pvap = ctx.enter_context(tc.tile_pool(name="pva", bufs=1, space="PSUM"))
        pva0 = pvap.tile([128, 4, H, 16], f32, tag="pva0")
        pva1 = pvap.tile([128, 4, H, 16], f32, tag="pva1")

        def qk_block(woff, srcs, td, sb, bounce_eng, scat_eng, tgt,
                     split=False):
            # split=True: two 4-dc psum pins with a DVE combine, so the
            # scores ring is never blocked for more than ~1.7us
            s80h = _s80h.pop(sb, None) if split else None
            dcs = range(4, ndc) if split else range(ndc)
            pq = psp.tile([80, 2, 512], f32, tag="ps")
            for dc in dcs:
                rhs = srcs[:, dc, sb * 512:(sb + 1) * 512]
                nc.tensor.matmul(pq[:, 0, :],
                                 lhsT=wall[:, dc, woff:woff + 80], rhs=rhs,
                                 start=(dc == dcs[0]), stop=(dc == ndc - 1))
                nc.tensor.matmul(pq[:, 1, :],
                                 lhsT=wall[:, dc, woff + 80:woff + 160], rhs=rhs,
                                 start=(dc == dcs[0]), stop=(dc == ndc - 1))
            s80 = s80p.tile([80, 2, 512], f16, tag="s80")
            if split:
                nc.vector.tensor_tensor(out=s80[:], in0=pq[:], in1=s80h[:],
                                        op=mybir.AluOpType.add)
            else:
                nc.vector.tensor_copy(out=s80[:], in_=pq[:])
            tda = td[:]
            rs = tda.ap[0][0]
            dst = bass.AP(
                tensor=tda.tensor, offset=tda.offset + sb * 512,
                ap=[[rs, 80], [rs * 80, 2], [1, 512]],
            )
            bounce_eng.dma_start(out=dst, in_=s80[:])
            if scat_eng is not None:
                scatter(scat_eng, td, tgt, sb * 512, (sb + 1) * 512)

        _s80h = {}

        def qk_half(woff, srcs, td, sb):
            pq = psp.tile([80, 2, 512], f32, tag="ps")
            for dc in range(4):
                rhs = srcs[:, dc, sb * 512:(sb + 1) * 512]
                nc.tensor.matmul(pq[:, 0, :],
                                 lhsT=wall[:, dc, woff:woff + 80], rhs=rhs,
                                 start=(dc == 0), stop=(dc == 3))
                nc.tensor.matmul(pq[:, 1, :],
                                 lhsT=wall[:, dc, woff + 80:woff + 160],
                                 rhs=rhs, start=(dc == 0), stop=(dc == 3))
            s80h = s80p.tile([80, 2, 512], f32, tag="s80h")
            nc.vector.tensor_copy(out=s80h[:], in_=pq[:])
            _s80h[sb] = s80h

        def v_step(tch, split=False):
            vn = psp.tile([128, H * DV], f32, tag="ps")
            for dc in range(ndc):
                nc.tensor.matmul(
                    vn[:],
                    lhsT=vtl[:, dc, tch * 128:(tch + 1) * 128],
                    rhs=wall[:, dc, 320:512],
                    start=(dc == 0), stop=(dc == ndc - 1),
                )
            nc.vector.tensor_copy(
                out=vex[:, tch, :, 0:DV],
                in_=vn[:].rearrange("p (h e) -> p h e", e=DV),
            )

        # PE warmup: idn self-transposes keep the PE continuously busy from
        # ~1us so the pstate is at max when the projections start
        for _ in range(64):
            wrm = psp.tile([128, 128], bf16, tag="ps")
            nc.tensor.transpose(wrm[:], idn[:], idn[:])

        # lead: Q (both blocks) + K block 0 + V0; scatters on scalar queue
        qk_block(0, qtl, qTdr, 0, nc.scalar, None, qT)
        qk_block(0, qtl, qTdr, 1, nc.scalar, None, qT)
        scatter(nc.scalar, qTdr, qT, 0, SH)
        qk_block(160, ktl, kTdr, 0, nc.scalar, nc.scalar, kT)
        # chain the bulk loads behind the K0 scatter: a marker copy reads
        # kT (produced by the scatter) into each bulk dest region, and the
        # bulk DMA's WAW dependency on the marker keeps the transfer FIFO
        # clear for the whole lead path
        for c0 in range(512, 2048, 256):
            nc.vector.tensor_copy(out=ktl[0:1, 0, c0:c0 + 1],
                                  in_=kT[0:1, 0, 0:1])
            nc.gpsimd.dma_start(out=ktl[:, :, c0:c0 + 256],
                                in_=KTr[:, :, c0:c0 + 256])
        for c0 in range(512, 2048, 256):
            nc.vector.tensor_copy(out=vtl[0:1, 0, c0:c0 + 1],
                                  in_=kT[0:1, 0, 0:1])
            nc.gpsimd.dma_start(out=vtl[:, :, c0:c0 + 256],
                                in_=VTr[:, :, c0:c0 + 256])
        v_step(0, split=False)

        # remaining setup interleaved into the attention loop (emission
        # deadlines: vex[t] before PV(t) emission, kT block b before
        # scores of tch 4b)
        tasks = [lambda: v_step(1),
                 lambda: qk_half(160, ktl, kTdr, 1),
                 lambda: qk_block(160, ktl, kTdr, 1, nc.gpsimd, nc.gpsimd, kT,
                                  split=True),
                 lambda: v_step(2),
                 lambda: v_step(3),
                 lambda: qk_half(160, ktl, kTdr, 2),
                 lambda: qk_block(160, ktl, kTdr, 2, nc.gpsimd, nc.gpsimd, kT,
                                  split=True),
                 lambda: v_step(4),
                 lambda: v_step(5),
                 lambda: qk_half(160, ktl, kTdr, 3),
                 lambda: qk_block(160, ktl, kTdr, 3, nc.gpsimd, nc.gpsimd, kT,
                                  split=True)]
        for t in range(6, ntc):
            tasks.append(lambda t=t: v_step(t))
        ti = 0

        def emit_pv(ex, h, tch):
            for sc in range(nsc):
                pva = pva0 if sc < 4 else pva1
                # psum start/stop are BANK-granular (2KB zero regions):
                # exactly one start (first write) and one stop (last write)
                # per sc-pair bank
                nc.tensor.matmul(
                    pva[:, sc % 4, h, 0:DV + 1],
                    lhsT=ex[:, sc * 128:(sc + 1) * 128],
                    rhs=vex[:, tch, h, :],
                    start=(tch == 0 and h == 0 and sc % 2 == 0),
                    stop=(tch == ntc - 1 and h == H - 1 and sc % 2 == 1),
                )

        prev = None
        for tch in range(ntc):
            for h in range(H):
                kb, kc = 32 * (h % 3), h // 3
                ps = psp.tile([128, SH], f32, tag="ps")
                for j in range(2):
                    nc.tensor.matmul(
                        ps[:, j * 512:(j + 1) * 512],
                        lhsT=kT[kb:kb + DK, kc, tch * 128:(tch + 1) * 128],
                        rhs=qT[kb:kb + DK, kc, j * 512:(j + 1) * 512],
                        start=True, stop=True,
                    )
                if prev is not None:
                    emit_pv(*prev)
                ex = exp_.tile([128, SH], bf16, tag="ex")
                nc.scalar.activation(out=ex[:], in_=ps[:], func=AF.Exp,
                                     scale=scale)
                prev = (ex, h, tch)
                if h in (7, 15) and ti < len(tasks):
                    tasks[ti]()
                    ti += 1
        emit_pv(*prev)

        # ---- epilogue: phase 1 normalizes + transposes all s-chunks
        # (DVE/PE), phase 2 runs WO matmuls with py in the freed pva slots
        asts = []
        for sc in range(nsc):
            pva = pva0 if sc < 4 else pva1
            rz = rzp.tile([128, H], f32, tag="rz")
            nc.vector.reciprocal(out=rz[:], in_=pva[:, sc % 4, :, DV])
            an = anp.tile([128, H * DV], bf16, tag="an")
            rzap = rz[:]
            rzb = bass.AP(
                tensor=rzap.tensor, offset=rzap.offset,
                ap=[rzap.ap[0], rzap.ap[1], [0, DV]],
            )
            nc.vector.tensor_tensor(
                out=an[:].rearrange("p (h e) -> p h e", e=DV),
                in0=pva[:, sc % 4, :, 0:DV],
                in1=rzb,
                op=mybir.AluOpType.mult,
            )
            aT = psp.tile([128, 256], bf16, tag="ps")
            nc.tensor.transpose(aT[:, 0:128], an[:, 0:128], idn[:])
            nc.tensor.transpose(aT[0:64, 128:256], an[:, 128:192], idn[:])
            ast = astp.tile([128, 256], bf16, tag="ast")
            nc.vector.tensor_copy(out=ast[:], in_=aT[:])
            asts.append(ast)
        for sc in range(nsc):
            py = pvap.tile([128, 2, 512], f32,
                           tag="pva0" if sc % 2 else "pva1")
            for db in range(2):
                nc.tensor.matmul(
                    py[:, db, :], lhsT=asts[sc][:, 0:128],
                    rhs=wosA[:, db * 512:(db + 1) * 512],
                    start=True, stop=False,
                )
                nc.tensor.matmul(
                    py[:, db, :], lhsT=asts[sc][0:64, 128:256],
                    rhs=wosB[:, db * 512:(db + 1) * 512],
                    start=False, stop=True,
                )
            yt = ytp.tile([128, 2, 512], f32, tag="yt")
            if sc % 2:
                nc.scalar.copy(out=yt[:], in_=py[:])
            else:
                nc.vector.tensor_copy(out=yt[:], in_=py[:])
            nc.sync.dma_start(
                out=Yd[sc * 128:(sc + 1) * 128, :],
                in_=yt[:].rearrange("p a b -> p (a b)"),
            )

    nc.compile()
    return nc


def _get_nc():
    if "nc" not in _NC_CACHE:
        _NC_CACHE["nc"] = _build_program()
    return _NC_CACHE["nc"]


def make_in_maps(Q, K, V, WQ, WK, WV, WO):
    import ml_dtypes

    bf = ml_dtypes.bfloat16
    f16 = np.float16
    wq = WQ.transpose(1, 0, 2).reshape(D, H * DK)
    wk = WK.transpose(1, 0, 2).reshape(D, H * DK)
    wv = WV.transpose(1, 0, 2).reshape(D, H * DV)
    wall = np.ascontiguousarray(
        np.concatenate([wq, wk, wv], axis=1)).astype(f16)
    woa = np.ascontiguousarray(WO[0:128, :]).astype(bf)
    wob = np.ascontiguousarray(WO[128:192, :]).astype(bf)
    idn = np.eye(128, dtype=bf)
    in_maps = []
    for c in range(8):
        b, g = c // 2, c % 2
        in_maps.append({
            "QT": np.ascontiguousarray(Q[b, g * SH:(g + 1) * SH, :].T).astype(f16),
            "KT": np.ascontiguousarray(K[b].T).astype(f16),
            "VT": np.ascontiguousarray(V[b].T).astype(f16),
            "WALL": wall,
            "WOA": woa, "WOB": wob, "IDN": idn,
        })
    return in_maps


LAST_RESULTS = None


def kernel(Q, K, V, WQ, WK, WV, WO, _trace=False):
    global LAST_RESULTS
    from concourse.bass_utils import run_bass_kernel_spmd

    Q = np.asarray(Q)
    K = np.asarray(K)
    V = np.asarray(V)
    nc = _get_nc()
    in_maps = make_in_maps(Q, K, V, np.asarray(WQ), np.asarray(WK),
                           np.asarray(WV), np.asarray(WO))
    res = run_bass_kernel_spmd(nc, in_maps, list(range(8)), trace=_trace)
    LAST_RESULTS = res
    out = np.empty((B, S, D), np.float32)
    for b in range(B):
        out[b, 0:SH] = res.results[2 * b]["Y"]
        out[b, SH:S] = res.results[2 * b + 1]["Y"]
    return out


# revision 42
# speedup vs baseline: 1.0022x; 1.0022x over previous
"""Trainium2 Bass kernel for nn_MultiHeadAttention_46213848104966.

B=4, S=2048, D=1024, H=16, DK=10, DV=12.
Sharding: 8 cores = 4 batches x 2 q-row halves; each core computes complete
output rows Y[b, half] over all 16 heads; the host concatenates.

The ScalarE exp stream (256 x [128,1024] tiles ~= 266us) is the hard floor;
everything else is arranged to hide underneath it:
  - host passes Q/K/V pre-transposed ([D, S] fp16) + packed fp16/bf16
    weights, so there are no on-device transposes or stage copies.
  - q/k projections (fp32 psum, fp16 in) bounce through DRAM to scatter
    into 32-partition-aligned per-head slots (3 m-group DMAs per tensor
    block); v projects into vex [t, h, 13] bf16 with a ones column.
  - scoresT = kT_h^T q_h (fp16 operands, fp32 psum), exp -> ex bf16; the
    PV matmul uses ex as the STATIONARY operand (weight load is free in
    the PE) streaming vex's 13 columns; pva[s, h, 13] accumulates over
    t, col 12 = Z.
  - PSUM phasing: scores ring (8KB) + K1-3/V proj psum (5.5KB) coexist;
    pva (8KB) opens once projections drain, so attention starts ~12us in
    while setup finishes; PV for tch 0-2 is emitted as a backlog after
    setup (ex ring is deep enough for Act to run ahead).
  - epilogue per s-chunk: 1/Z (DVE), fused normalize-mul -> an bf16,
    PE-transpose, WO matmul, Y out.
"""

import numpy as np
from contextlib import ExitStack

S = 2048
SH = 1024  # q rows per core
D = 1024
H = 16
DK = 10
DV = 12
B = 4

_NC_CACHE = {}


def _build_program():
    import concourse.bass as bass
    import concourse.tile as tile
    from concourse import bacc, mybir

    f32 = mybir.dt.float32
    f16 = mybir.dt.float16
    bf16 = mybir.dt.bfloat16
    AF = mybir.ActivationFunctionType

    ndc = D // 128            # 8 d-chunks
    ntc = S // 128            # 16 t-chunks
    nsc = SH // 128           # 8 s-chunks
    NDEFER = 3                # t-chunks whose PV is emitted after setup

    nc = bacc.Bacc("TRN2", target_bir_lowering=False, debug=False, num_devices=8)

    QTd = nc.dram_tensor("QT", [D, SH], f16, kind="ExternalInput").ap()
    KTd = nc.dram_tensor("KT", [D, S], f16, kind="ExternalInput").ap()
    VTd = nc.dram_tensor("VT", [D, S], f16, kind="ExternalInput").ap()
    WALLd = nc.dram_tensor("WALL", [D, 512], f16, kind="ExternalInput").ap()
    WOAd = nc.dram_tensor("WOA", [128, D], bf16, kind="ExternalInput").ap()
    WOBd = nc.dram_tensor("WOB", [64, D], bf16, kind="ExternalInput").ap()
    IDd = nc.dram_tensor("IDN", [128, 128], bf16, kind="ExternalInput").ap()
    Yd = nc.dram_tensor("Y", [SH, D], f32, kind="ExternalOutput").ap()

    scale = float(np.float32(1.0) / np.sqrt(np.float32(10.0)))

    with tile.TileContext(nc) as tc, ExitStack() as ctx:
        consts = ctx.enter_context(tc.tile_pool(name="consts", bufs=1))
        qkvp = ctx.enter_context(tc.tile_pool(name="qkv", bufs=1))
        s80p = ctx.enter_context(tc.tile_pool(name="s80", bufs=2))
        exp_ = ctx.enter_context(tc.tile_pool(name="ex", bufs=8))
        anp = ctx.enter_context(tc.tile_pool(name="an", bufs=4))
        astp = ctx.enter_context(tc.tile_pool(name="ast", bufs=8))
        rzp = ctx.enter_context(tc.tile_pool(name="rz", bufs=2))
        ytp = ctx.enter_context(tc.tile_pool(name="yt", bufs=3))
        dramp = ctx.enter_context(tc.tile_pool(name="od", bufs=1, space="DRAM"))

        idn = consts.tile([128, 128], bf16, tag="idn")
        nc.sync.dma_start(out=idn[:], in_=IDd)
        wall = consts.tile([128, ndc, 512], f16, tag="wall")
        WALLr = WALLd.rearrange("(c p) m -> p c m", p=128)
        nc.sync.dma_start(out=wall[:, :, 0:160], in_=WALLr[:, :, 0:160])
        # (wk loads just before ktl0; wv + vtl0 are marker-chained after
        # the K0 scatter so the lead transfer FIFO stays clear)
        wosA = consts.tile([128, D], bf16, tag="wosA")
        nc.gpsimd.dma_start(out=wosA[:], in_=WOAd)
        wosB = consts.tile([64, D], bf16, tag="wosB")
        nc.gpsimd.dma_start(out=wosB[:], in_=WOBd)

        # per-dc slabs of the host-transposed tensors
        qtl = consts.tile([128, ndc, SH], f16, tag="qtl")
        ktl = consts.tile([128, ndc, S], f16, tag="ktl")
        vtl = consts.tile([128, ndc, S], f16, tag="vtl")
        # lead-path loads first (one DMA each): Q fully, K block-0 cols,
        # V t-chunks 0-3 cols; the bulk arrives while attention runs
        QTr = QTd.rearrange("(c p) s -> p c s", p=128)
        KTr = KTd.rearrange("(c p) s -> p c s", p=128)
        VTr = VTd.rearrange("(c p) s -> p c s", p=128)
        nc.sync.dma_start(out=qtl[:, :, 0:512], in_=QTr[:, :, 0:512])
        nc.sync.dma_start(out=qtl[:, :, 512:1024], in_=QTr[:, :, 512:1024])
        nc.sync.dma_start(out=ktl[:, :, 0:512], in_=KTr[:, :, 0:512])
        nc.sync.dma_start(out=wall[:, :, 160:512], in_=WALLr[:, :, 160:512])
        nc.sync.dma_start(out=vtl[:, :, 0:512], in_=VTr[:, :, 0:512])
        # bulk pieces carry a late scheduler priority so every lead-path
        # DMA beats them into the transfer FIFO; data deps still pull each
        # piece in before its first consumer

        # head h -> partitions 32*(h%3)..+10 of chunk h//3
        kT = qkvp.tile([128, 6, S], f16, tag="kT")
        qT = qkvp.tile([128, 6, SH], f16, tag="qT")
        vex = qkvp.tile([128, ntc, H, DV + 1], bf16, tag="vex")
        nc.vector.memset(vex[:, :, :, DV], 1.0)

        qTdr = dramp.tile([H * DK, SH], f16, tag="qTdr")
        kTdr = dramp.tile([H * DK, S], f16, tag="kTdr")

        def scatter(dma_eng, td, tgt, c0, c1):
            # src rows 30c'+10m+k -> dest partitions 32m+k, chunk c'
            w = c1 - c0
            for m in range(3):
                nch = 6 if m == 0 else 5
                tda = td[:]
                src = bass.AP(
                    tensor=tda.tensor,
                    offset=tda.offset + (10 * m) * tda.ap[0][0] + c0,
                    ap=[[tda.ap[0][0], DK], [30 * tda.ap[0][0], nch], [1, w]],
                )
                dma_eng.dma_start(
                    out=tgt[32 * m:32 * m + DK, 0:nch, c0:c1], in_=src)

        # ---------------- psum pools ----------------
        # ps ring (8KB) + pva (8KB) fill PSUM; K1-3/V projection psum
        # borrows ps-ring slots (tag "ps") so everything coexists.
        psp = ctx.enter_context(tc.tile_pool(name="ps", bufs=2, space="PSUM"))
        pvap = ctx.enter_context(tc.tile_pool(name="pva", bufs=1, space="PSUM"))
        pva0 = pvap.tile([128, 4, H, 16], f32, tag="pva0")
        pva1 = pvap.tile([128, 4, H, 16], f32, tag="pva1")

        def qk_block(woff, srcs, td, sb, bounce_eng, scat_eng, tgt,
                     split=False):
            # split=True: two 4-dc psum pins with a DVE combine, so the
            # scores ring is never blocked for more than ~1.7us
            s80h = _s80h.pop(sb, None) if split else None
            dcs = range(4, ndc) if split else range(ndc)
            pq = psp.tile([80, 2, 512], f32, tag="ps")
            for dc in dcs:
                rhs = srcs[:, dc, sb * 512:(sb + 1) * 512]
                nc.tensor.matmul(pq[:, 0, :],
                                 lhsT=wall[:, dc, woff:woff + 80], rhs=rhs,
                                 start=(dc == dcs[0]), stop=(dc == ndc - 1))
                nc.tensor.matmul(pq[:, 1, :],
                                 lhsT=wall[:, dc, woff + 80:woff + 160], rhs=rhs,
                                 start=(dc == dcs[0]), stop=(dc == ndc - 1))
            s80 = s80p.tile([80, 2, 512], f16, tag="s80")
            if split:
                nc.vector.tensor_tensor(out=s80[:], in0=pq[:], in1=s80h[:],
                                        op=mybir.AluOpType.add)
            else:
                nc.vector.tensor_copy(out=s80[:], in_=pq[:])
            tda = td[:]
            rs = tda.ap[0][0]
            dst = bass.AP(
                tensor=tda.tensor, offset=tda.offset + sb * 512,
                ap=[[rs, 80], [rs * 80, 2], [1, 512]],
            )
            bounce_eng.dma_start(out=dst, in_=s80[:])
            if scat_eng is not None:
                scatter(scat_eng, td, tgt, sb * 512, (sb + 1) * 512)

        _s80h = {}

        def qk_half(woff, srcs, td, sb):
            pq = psp.tile([80, 2, 512], f32, tag="ps")
            for dc in range(4):
                rhs = srcs[:, dc, sb * 512:(sb + 1) * 512]
                nc.tensor.matmul(pq[:, 0, :],
                                 lhsT=wall[:, dc, woff:woff + 80], rhs=rhs,
                                 start=(dc == 0), stop=(dc == 3))
                nc.tensor.matmul(pq[:, 1, :],
                                 lhsT=wall[:, dc, woff + 80:woff + 160],
                                 rhs=rhs, start=(dc == 0), stop=(dc == 3))
            s80h = s80p.tile([80, 2, 512], f32, tag="s80h")
            nc.vector.tensor_copy(out=s80h[:], in_=pq[:])
            _s80h[sb] = s80h

        def v_step(tch, split=False):
            vn = psp.tile([128, H * DV], f32, tag="ps")
            for dc in range(ndc):
                nc.tensor.matmul(
                    vn[:],
                    lhsT=vtl[:, dc, tch * 128:(tch + 1) * 128],
                    rhs=wall[:, dc, 320:512],
                    start=(dc == 0), stop=(dc == ndc - 1),
                )
            nc.vector.tensor_copy(
                out=vex[:, tch, :, 0:DV],
                in_=vn[:].rearrange("p (h e) -> p h e", e=DV),
            )

        # PE warmup: idn self-transposes keep the PE continuously busy from
        # ~1us so the pstate is at max when the projections start
        for _ in range(48):
            wrm = psp.tile([128, 128], bf16, tag="ps")
            nc.tensor.transpose(wrm[:], idn[:], idn[:])

        # lead: Q (both blocks) + K block 0 + V0; scatters on scalar queue
        qk_block(0, qtl, qTdr, 0, nc.scalar, None, qT)
        qk_block(0, qtl, qTdr, 1, nc.scalar, None, qT)
        scatter(nc.scalar, qTdr, qT, 0, SH)
        qk_block(160, ktl, kTdr, 0, nc.scalar, nc.scalar, kT)
        # chain the bulk loads behind the K0 scatter: a marker copy reads
        # kT (produced by the scatter) into each bulk dest region, and the
        # bulk DMA's WAW dependency on the marker keeps the transfer FIFO
        # clear for the whole lead path
        for c0 in range(512, 2048, 256):
            nc.vector.tensor_copy(out=ktl[0:1, 0, c0:c0 + 1],
                                  in_=kT[0:1, 0, 0:1])
            nc.gpsimd.dma_start(out=ktl[:, :, c0:c0 + 256],
                                in_=KTr[:, :, c0:c0 + 256])
        for c0 in range(512, 2048, 256):
            nc.vector.tensor_copy(out=vtl[0:1, 0, c0:c0 + 1],
                                  in_=kT[0:1, 0, 0:1])
            nc.gpsimd.dma_start(out=vtl[:, :, c0:c0 + 256],
                                in_=VTr[:, :, c0:c0 + 256])
        v_step(0, split=False)

        # remaining setup interleaved into the attention loop (emission
        # deadlines: vex[t] before PV(t) emission, kT block b before
        # scores of tch 4b)
        tasks = [lambda: v_step(1),
                 lambda: qk_half(160, ktl, kTdr, 1),
                 lambda: qk_block(160, ktl, kTdr, 1, nc.gpsimd, nc.gpsimd, kT,
                                  split=True),
                 lambda: v_step(2),
                 lambda: v_step(3),
                 lambda: qk_half(160, ktl, kTdr, 2),
                 lambda: qk_block(160, ktl, kTdr, 2, nc.gpsimd, nc.gpsimd, kT,
                                  split=True),
                 lambda: v_step(4),
                 lambda: v_step(5),
                 lambda: qk_half(160, ktl, kTdr, 3),
                 lambda: qk_block(160, ktl, kTdr, 3, nc.gpsimd, nc.gpsimd, kT,
                                  split=True)]
        for t in range(6, ntc):
            tasks.append(lambda t=t: v_step(t))
        ti = 0

        def emit_pv(ex, h, tch):
            for sc in range(nsc):
                pva = pva0 if sc < 4 else pva1
                # psum start/stop are BANK-granular (2KB zero regions):
                # exactly one start (first write) and one stop (last write)
                # per sc-pair bank
                nc.tensor.matmul(
                    pva[:, sc % 4, h, 0:DV + 1],
                    lhsT=ex[:, sc * 128:(sc + 1) * 128],
                    rhs=vex[:, tch, h, :],
                    start=(tch == 0 and h == 0 and sc % 2 == 0),
                    stop=(tch == ntc - 1 and h == H - 1 and sc % 2 == 1),
                )

        prev = None
        for tch in range(ntc):
            for h in range(H):
                kb, kc = 32 * (h % 3), h // 3
                ps = psp.tile([128, SH], f32, tag="ps")
                for j in range(2):
                    nc.tensor.matmul(
                        ps[:, j * 512:(j + 1) * 512],
                        lhsT=kT[kb:kb + DK, kc, tch * 128:(tch + 1) * 128],
                        rhs=qT[kb:kb + DK, kc, j * 512:(j + 1) * 512],
                        start=True, stop=True,
                    )
                if prev is not None:
                    emit_pv(*prev)
                ex = exp_.tile([128, SH], bf16, tag="ex")
                nc.scalar.activation(out=ex[:], in_=ps[:], func=AF.Exp,
                                     scale=scale)
                prev = (ex, h, tch)
                if h in (7, 15) and ti < len(tasks):
                    tasks[ti]()
                    ti += 1
        emit_pv(*prev)

        # ---- epilogue: phase 1 normalizes + transposes all s-chunks
        # (DVE/PE), phase 2 runs WO matmuls with py in the freed pva slots
        asts = []
        for sc in range(nsc):
            pva = pva0 if sc < 4 else pva1
            rz = rzp.tile([128, H], f32, tag="rz")
            nc.vector.reciprocal(out=rz[:], in_=pva[:, sc % 4, :, DV])
            an = anp.tile([128, H * DV], bf16, tag="an")
            rzap = rz[:]
            rzb = bass.AP(
                tensor=rzap.tensor, offset=rzap.offset,
                ap=[rzap.ap[0], rzap.ap[1], [0, DV]],
            )
            nc.vector.tensor_tensor(
                out=an[:].rearrange("p (h e) -> p h e", e=DV),
                in0=pva[:, sc % 4, :, 0:DV],
                in1=rzb,
                op=mybir.AluOpType.mult,
            )
            aT = psp.tile([128, 256], bf16, tag="ps")
            nc.tensor.transpose(aT[:, 0:128], an[:, 0:128], idn[:])
            nc.tensor.transpose(aT[0:64, 128:256], an[:, 128:192], idn[:])
            ast = astp.tile([128, 256], bf16, tag="ast")
            nc.vector.tensor_copy(out=ast[:], in_=aT[:])
            asts.append(ast)
        for sc in range(nsc):
            py = pvap.tile([128, 2, 512], f32,
                           tag="pva0" if sc % 2 else "pva1")
            for db in range(2):
                nc.tensor.matmul(
                    py[:, db, :], lhsT=asts[sc][:, 0:128],
                    rhs=wosA[:, db * 512:(db + 1) * 512],
                    start=True, stop=False,
                )
                nc.tensor.matmul(
                    py[:, db, :], lhsT=asts[sc][0:64, 128:256],
                    rhs=wosB[:, db * 512:(db + 1) * 512],
                    start=False, stop=True,
                )
            yt = ytp.tile([128, 2, 512], f32, tag="yt")
            if sc % 2:
                nc.scalar.copy(out=yt[:], in_=py[:])
            else:
                nc.vector.tensor_copy(out=yt[:], in_=py[:])
            nc.sync.dma_start(
                out=Yd[sc * 128:(sc + 1) * 128, :],
                in_=yt[:].rearrange("p a b -> p (a b)"),
            )

    nc.compile()
    return nc


def _get_nc():
    if "nc" not in _NC_CACHE:
        _NC_CACHE["nc"] = _build_program()
    return _NC_CACHE["nc"]


def make_in_maps(Q, K, V, WQ, WK, WV, WO):
    import ml_dtypes

    bf = ml_dtypes.bfloat16
    f16 = np.float16
    wq = WQ.transpose(1, 0, 2).reshape(D, H * DK)
    wk = WK.transpose(1, 0, 2).reshape(D, H * DK)
    wv = WV.transpose(1, 0, 2).reshape(D, H * DV)
    wall = np.ascontiguousarray(
        np.concatenate([wq, wk, wv], axis=1)).astype(f16)
    woa = np.ascontiguousarray(WO[0:128, :]).astype(bf)
    wob = np.ascontiguousarray(WO[128:192, :]).astype(bf)
    idn = np.eye(128, dtype=bf)
    in_maps = []
    for c in range(8):
        b, g = c // 2, c % 2
        in_maps.append({
            "QT": np.ascontiguousarray(Q[b, g * SH:(g + 1) * SH, :].T).astype(f16),
            "KT": np.ascontiguousarray(K[b].T).astype(f16),
            "VT": np.ascontiguousarray(V[b].T).astype(f16),
            "WALL": wall,
            "WOA": woa, "WOB": wob, "IDN": idn,
        })
    return in_maps


LAST_RESULTS = None


def kernel(Q, K, V, WQ, WK, WV, WO, _trace=False):
    global LAST_RESULTS
    from concourse.bass_utils import run_bass_kernel_spmd

    Q = np.asarray(Q)
    K = np.asarray(K)
    V = np.asarray(V)
    nc = _get_nc()
    in_maps = make_in_maps(Q, K, V, np.asarray(WQ), np.asarray(WK),
                           np.asarray(WV), np.asarray(WO))
    res = run_bass_kernel_spmd(nc, in_maps, list(range(8)), trace=_trace)
    LAST_RESULTS = res
    out = np.empty((B, S, D), np.float32)
    for b in range(B):
        out[b, 0:SH] = res.results[2 * b]["Y"]
        out[b, SH:S] = res.results[2 * b + 1]["Y"]
    return out


# revision 43
# speedup vs baseline: 1.0043x; 1.0021x over previous
"""Trainium2 Bass kernel for nn_MultiHeadAttention_46213848104966.

B=4, S=2048, D=1024, H=16, DK=10, DV=12.
Sharding: 8 cores = 4 batches x 2 q-row halves; each core computes complete
output rows Y[b, half] over all 16 heads; the host concatenates.

The ScalarE exp stream (256 x [128,1024] tiles ~= 266us) is the hard floor;
everything else is arranged to hide underneath it:
  - host passes Q/K/V pre-transposed ([D, S] fp16) + packed fp16/bf16
    weights, so there are no on-device transposes or stage copies.
  - q/k projections (fp32 psum, fp16 in) bounce through DRAM to scatter
    into 32-partition-aligned per-head slots (3 m-group DMAs per tensor
    block); v projects into vex [t, h, 13] bf16 with a ones column.
  - scoresT = kT_h^T q_h (fp16 operands, fp32 psum), exp -> ex bf16; the
    PV matmul uses ex as the STATIONARY operand (weight load is free in
    the PE) streaming vex's 13 columns; pva[s, h, 13] accumulates over
    t, col 12 = Z.
  - PSUM phasing: scores ring (8KB) + K1-3/V proj psum (5.5KB) coexist;
    pva (8KB) opens once projections drain, so attention starts ~12us in
    while setup finishes; PV for tch 0-2 is emitted as a backlog after
    setup (ex ring is deep enough for Act to run ahead).
  - epilogue per s-chunk: 1/Z (DVE), fused normalize-mul -> an bf16,
    PE-transpose, WO matmul, Y out.
"""

import numpy as np
from contextlib import ExitStack

S = 2048
SH = 1024  # q rows per core
D = 1024
H = 16
DK = 10
DV = 12
B = 4

_NC_CACHE = {}


def _build_program():
    import concourse.bass as bass
    import concourse.tile as tile
    from concourse import bacc, mybir

    f32 = mybir.dt.float32
    f16 = mybir.dt.float16
    bf16 = mybir.dt.bfloat16
    AF = mybir.ActivationFunctionType

    ndc = D // 128            # 8 d-chunks
    ntc = S // 128            # 16 t-chunks
    nsc = SH // 128           # 8 s-chunks
    NDEFER = 3                # t-chunks whose PV is emitted after setup

    nc = bacc.Bacc("TRN2", target_bir_lowering=False, debug=False, num_devices=8)

    QTd = nc.dram_tensor("QT", [D, SH], f16, kind="ExternalInput").ap()
    KTd = nc.dram_tensor("KT", [D, S], f16, kind="ExternalInput").ap()
    VTd = nc.dram_tensor("VT", [D, S], f16, kind="ExternalInput").ap()
    WALLd = nc.dram_tensor("WALL", [D, 512], f16, kind="ExternalInput").ap()
    WOAd = nc.dram_tensor("WOA", [128, D], bf16, kind="ExternalInput").ap()
    WOBd = nc.dram_tensor("WOB", [64, D], bf16, kind="ExternalInput").ap()
    IDd = nc.dram_tensor("IDN", [128, 128], bf16, kind="ExternalInput").ap()
    Yd = nc.dram_tensor("Y", [SH, D], f32, kind="ExternalOutput").ap()

    scale = float(np.float32(1.0) / np.sqrt(np.float32(10.0)))

    with tile.TileContext(nc) as tc, ExitStack() as ctx:
        consts = ctx.enter_context(tc.tile_pool(name="consts", bufs=1))
        qkvp = ctx.enter_context(tc.tile_pool(name="qkv", bufs=1))
        s80p = ctx.enter_context(tc.tile_pool(name="s80", bufs=2))
        exp_ = ctx.enter_context(tc.tile_pool(name="ex", bufs=8))
        anp = ctx.enter_context(tc.tile_pool(name="an", bufs=4))
        astp = ctx.enter_context(tc.tile_pool(name="ast", bufs=8))
        rzp = ctx.enter_context(tc.tile_pool(name="rz", bufs=2))
        ytp = ctx.enter_context(tc.tile_pool(name="yt", bufs=4))
        dramp = ctx.enter_context(tc.tile_pool(name="od", bufs=1, space="DRAM"))

        idn = consts.tile([128, 128], bf16, tag="idn")
        nc.sync.dma_start(out=idn[:], in_=IDd)
        wall = consts.tile([128, ndc, 512], f16, tag="wall")
        WALLr = WALLd.rearrange("(c p) m -> p c m", p=128)
        nc.sync.dma_start(out=wall[:, :, 0:160], in_=WALLr[:, :, 0:160])
        # (wk loads just before ktl0; wv + vtl0 are marker-chained after
        # the K0 scatter so the lead transfer FIFO stays clear)
        wosA = consts.tile([128, D], bf16, tag="wosA")
        nc.gpsimd.dma_start(out=wosA[:], in_=WOAd)
        wosB = consts.tile([64, D], bf16, tag="wosB")
        nc.gpsimd.dma_start(out=wosB[:], in_=WOBd)

        # per-dc slabs of the host-transposed tensors
        qtl = consts.tile([128, ndc, SH], f16, tag="qtl")
        ktl = consts.tile([128, ndc, S], f16, tag="ktl")
        vtl = consts.tile([128, ndc, S], f16, tag="vtl")
        # lead-path loads first (one DMA each): Q fully, K block-0 cols,
        # V t-chunks 0-3 cols; the bulk arrives while attention runs
        QTr = QTd.rearrange("(c p) s -> p c s", p=128)
        KTr = KTd.rearrange("(c p) s -> p c s", p=128)
        VTr = VTd.rearrange("(c p) s -> p c s", p=128)
        nc.sync.dma_start(out=qtl[:, :, 0:512], in_=QTr[:, :, 0:512])
        nc.sync.dma_start(out=qtl[:, :, 512:1024], in_=QTr[:, :, 512:1024])
        nc.sync.dma_start(out=ktl[:, :, 0:512], in_=KTr[:, :, 0:512])
        nc.sync.dma_start(out=wall[:, :, 160:512], in_=WALLr[:, :, 160:512])
        nc.sync.dma_start(out=vtl[:, :, 0:512], in_=VTr[:, :, 0:512])
        # bulk pieces carry a late scheduler priority so every lead-path
        # DMA beats them into the transfer FIFO; data deps still pull each
        # piece in before its first consumer

        # head h -> partitions 32*(h%3)..+10 of chunk h//3
        kT = qkvp.tile([128, 6, S], f16, tag="kT")
        qT = qkvp.tile([128, 6, SH], f16, tag="qT")
        vex = qkvp.tile([128, ntc, H, DV + 1], bf16, tag="vex")
        nc.vector.memset(vex[:, :, :, DV], 1.0)

        qTdr = dramp.tile([H * DK, SH], f16, tag="qTdr")
        kTdr = dramp.tile([H * DK, S], f16, tag="kTdr")

        def scatter(dma_eng, td, tgt, c0, c1):
            # src rows 30c'+10m+k -> dest partitions 32m+k, chunk c'
            w = c1 - c0
            for m in range(3):
                nch = 6 if m == 0 else 5
                tda = td[:]
                src = bass.AP(
                    tensor=tda.tensor,
                    offset=tda.offset + (10 * m) * tda.ap[0][0] + c0,
                    ap=[[tda.ap[0][0], DK], [30 * tda.ap[0][0], nch], [1, w]],
                )
                dma_eng.dma_start(
                    out=tgt[32 * m:32 * m + DK, 0:nch, c0:c1], in_=src)

        # ---------------- psum pools ----------------
        # ps ring (8KB) + pva (8KB) fill PSUM; K1-3/V projection psum
        # borrows ps-ring slots (tag "ps") so everything coexists.
        psp = ctx.enter_context(tc.tile_pool(name="ps", bufs=2, space="PSUM"))
        pvap = ctx.enter_context(tc.tile_pool(name="pva", bufs=1, space="PSUM"))
        pva0 = pvap.tile([128, 4, H, 16], f32, tag="pva0")
        pva1 = pvap.tile([128, 4, H, 16], f32, tag="pva1")

        def qk_block(woff, srcs, td, sb, bounce_eng, scat_eng, tgt,
                     split=False):
            # split=True: two 4-dc psum pins with a DVE combine, so the
            # scores ring is never blocked for more than ~1.7us
            s80h = _s80h.pop(sb, None) if split else None
            dcs = range(4, ndc) if split else range(ndc)
            pq = psp.tile([80, 2, 512], f32, tag="ps")
            for dc in dcs:
                rhs = srcs[:, dc, sb * 512:(sb + 1) * 512]
                nc.tensor.matmul(pq[:, 0, :],
                                 lhsT=wall[:, dc, woff:woff + 80], rhs=rhs,
                                 start=(dc == dcs[0]), stop=(dc == ndc - 1))
                nc.tensor.matmul(pq[:, 1, :],
                                 lhsT=wall[:, dc, woff + 80:woff + 160], rhs=rhs,
                                 start=(dc == dcs[0]), stop=(dc == ndc - 1))
            s80 = s80p.tile([80, 2, 512], f16, tag="s80")
            if split:
                nc.vector.tensor_tensor(out=s80[:], in0=pq[:], in1=s80h[:],
                                        op=mybir.AluOpType.add)
            else:
                nc.vector.tensor_copy(out=s80[:], in_=pq[:])
            tda = td[:]
            rs = tda.ap[0][0]
            dst = bass.AP(
                tensor=tda.tensor, offset=tda.offset + sb * 512,
                ap=[[rs, 80], [rs * 80, 2], [1, 512]],
            )
            bounce_eng.dma_start(out=dst, in_=s80[:])
            if scat_eng is not None:
                scatter(scat_eng, td, tgt, sb * 512, (sb + 1) * 512)

        _s80h = {}

        def qk_half(woff, srcs, td, sb):
            pq = psp.tile([80, 2, 512], f32, tag="ps")
            for dc in range(4):
                rhs = srcs[:, dc, sb * 512:(sb + 1) * 512]
                nc.tensor.matmul(pq[:, 0, :],
                                 lhsT=wall[:, dc, woff:woff + 80], rhs=rhs,
                                 start=(dc == 0), stop=(dc == 3))
                nc.tensor.matmul(pq[:, 1, :],
                                 lhsT=wall[:, dc, woff + 80:woff + 160],
                                 rhs=rhs, start=(dc == 0), stop=(dc == 3))
            s80h = s80p.tile([80, 2, 512], f32, tag="s80h")
            nc.vector.tensor_copy(out=s80h[:], in_=pq[:])
            _s80h[sb] = s80h

        def v_step(tch, split=False):
            vn = psp.tile([128, H * DV], f32, tag="ps")
            for dc in range(ndc):
                nc.tensor.matmul(
                    vn[:],
                    lhsT=vtl[:, dc, tch * 128:(tch + 1) * 128],
                    rhs=wall[:, dc, 320:512],
                    start=(dc == 0), stop=(dc == ndc - 1),
                )
            nc.vector.tensor_copy(
                out=vex[:, tch, :, 0:DV],
                in_=vn[:].rearrange("p (h e) -> p h e", e=DV),
            )

        # PE warmup: idn self-transposes keep the PE continuously busy from
        # ~1us so the pstate is at max when the projections start
        for _ in range(48):
            wrm = psp.tile([128, 128], bf16, tag="ps")
            nc.tensor.transpose(wrm[:], idn[:], idn[:])

        # lead: Q (both blocks) + K block 0 + V0; scatters on scalar queue
        qk_block(0, qtl, qTdr, 0, nc.scalar, None, qT)
        qk_block(0, qtl, qTdr, 1, nc.scalar, None, qT)
        scatter(nc.scalar, qTdr, qT, 0, SH)
        qk_block(160, ktl, kTdr, 0, nc.scalar, nc.scalar, kT)
        # chain the bulk loads behind the K0 scatter: a marker copy reads
        # kT (produced by the scatter) into each bulk dest region, and the
        # bulk DMA's WAW dependency on the marker keeps the transfer FIFO
        # clear for the whole lead path
        for c0 in range(512, 2048, 256):
            nc.vector.tensor_copy(out=ktl[0:1, 0, c0:c0 + 1],
                                  in_=kT[0:1, 0, 0:1])
            nc.gpsimd.dma_start(out=ktl[:, :, c0:c0 + 256],
                                in_=KTr[:, :, c0:c0 + 256])
        for c0 in range(512, 2048, 256):
            nc.vector.tensor_copy(out=vtl[0:1, 0, c0:c0 + 1],
                                  in_=kT[0:1, 0, 0:1])
            nc.gpsimd.dma_start(out=vtl[:, :, c0:c0 + 256],
                                in_=VTr[:, :, c0:c0 + 256])
        v_step(0, split=False)

        # remaining setup interleaved into the attention loop (emission
        # deadlines: vex[t] before PV(t) emission, kT block b before
        # scores of tch 4b)
        tasks = [lambda: v_step(1),
                 lambda: qk_half(160, ktl, kTdr, 1),
                 lambda: qk_block(160, ktl, kTdr, 1, nc.gpsimd, nc.gpsimd, kT,
                                  split=True),
                 lambda: v_step(2),
                 lambda: v_step(3),
                 lambda: qk_half(160, ktl, kTdr, 2),
                 lambda: qk_block(160, ktl, kTdr, 2, nc.gpsimd, nc.gpsimd, kT,
                                  split=True),
                 lambda: v_step(4),
                 lambda: v_step(5),
                 lambda: qk_half(160, ktl, kTdr, 3),
                 lambda: qk_block(160, ktl, kTdr, 3, nc.gpsimd, nc.gpsimd, kT,
                                  split=True)]
        for t in range(6, ntc):
            tasks.append(lambda t=t: v_step(t))
        ti = 0

        def emit_pv(ex, h, tch):
            for sc in range(nsc):
                pva = pva0 if sc < 4 else pva1
                # psum start/stop are BANK-granular (2KB zero regions):
                # exactly one start (first write) and one stop (last write)
                # per sc-pair bank
                nc.tensor.matmul(
                    pva[:, sc % 4, h, 0:DV + 1],
                    lhsT=ex[:, sc * 128:(sc + 1) * 128],
                    rhs=vex[:, tch, h, :],
                    start=(tch == 0 and h == 0 and sc % 2 == 0),
                    stop=(tch == ntc - 1 and h == H - 1 and sc % 2 == 1),
                )

        prev = None
        for tch in range(ntc):
            for h in range(H):
                kb, kc = 32 * (h % 3), h // 3
                ps = psp.tile([128, SH], f32, tag="ps")
                for j in range(2):
                    nc.tensor.matmul(
                        ps[:, j * 512:(j + 1) * 512],
                        lhsT=kT[kb:kb + DK, kc, tch * 128:(tch + 1) * 128],
                        rhs=qT[kb:kb + DK, kc, j * 512:(j + 1) * 512],
                        start=True, stop=True,
                    )
                if prev is not None:
                    emit_pv(*prev)
                ex = exp_.tile([128, SH], bf16, tag="ex")
                nc.scalar.activation(out=ex[:], in_=ps[:], func=AF.Exp,
                                     scale=scale)
                prev = (ex, h, tch)
                if h in (7, 15) and ti < len(tasks):
                    tasks[ti]()
                    ti += 1
        emit_pv(*prev)

        # ---- epilogue: phase 1 normalizes + transposes all s-chunks
        # (DVE/PE), phase 2 runs WO matmuls with py in the freed pva slots
        asts = []
        for sc in range(nsc):
            pva = pva0 if sc < 4 else pva1
            rz = rzp.tile([128, H], f32, tag="rz")
            nc.vector.reciprocal(out=rz[:], in_=pva[:, sc % 4, :, DV])
            an = anp.tile([128, H * DV], bf16, tag="an")
            rzap = rz[:]
            rzb = bass.AP(
                tensor=rzap.tensor, offset=rzap.offset,
                ap=[rzap.ap[0], rzap.ap[1], [0, DV]],
            )
            nc.vector.tensor_tensor(
                out=an[:].rearrange("p (h e) -> p h e", e=DV),
                in0=pva[:, sc % 4, :, 0:DV],
                in1=rzb,
                op=mybir.AluOpType.mult,
            )
            aT = psp.tile([128, 256], bf16, tag="ps")
            nc.tensor.transpose(aT[:, 0:128], an[:, 0:128], idn[:])
            nc.tensor.transpose(aT[0:64, 128:256], an[:, 128:192], idn[:])
            ast = astp.tile([128, 256], bf16, tag="ast")
            nc.vector.tensor_copy(out=ast[:], in_=aT[:])
            asts.append(ast)
        for sc in range(nsc):
            py = pvap.tile([128, 2, 512], f32,
                           tag="pva0" if sc % 2 else "pva1")
            for db in range(2):
                nc.tensor.matmul(
                    py[:, db, :], lhsT=asts[sc][:, 0:128],
                    rhs=wosA[:, db * 512:(db + 1) * 512],
                    start=True, stop=False,
                )
                nc.tensor.matmul(
                    py[:, db, :], lhsT=asts[sc][0:64, 128:256],
                    rhs=wosB[:, db * 512:(db + 1) * 512],
                    start=False, stop=True,
                )
            yt = ytp.tile([128, 2, 512], f32, tag="yt")
            if sc % 2:
                nc.scalar.copy(out=yt[:], in_=py[:])
            else:
                nc.vector.tensor_copy(out=yt[:], in_=py[:])
            nc.sync.dma_start(
                out=Yd[sc * 128:(sc + 1) * 128, :],
                in_=yt[:].rearrange("p a b -> p (a b)"),
            )

    nc.compile()
    return nc


def _get_nc():
    if "nc" not in _NC_CACHE:
        _NC_CACHE["nc"] = _build_program()
    return _NC_CACHE["nc"]


def make_in_maps(Q, K, V, WQ, WK, WV, WO):
    import ml_dtypes

    bf = ml_dtypes.bfloat16
    f16 = np.float16
    wq = WQ.transpose(1, 0, 2).reshape(D, H * DK)
    wk = WK.transpose(1, 0, 2).reshape(D, H * DK)
    wv = WV.transpose(1, 0, 2).reshape(D, H * DV)
    wall = np.ascontiguousarray(
        np.concatenate([wq, wk, wv], axis=1)).astype(f16)
    woa = np.ascontiguousarray(WO[0:128, :]).astype(bf)
    wob = np.ascontiguousarray(WO[128:192, :]).astype(bf)
    idn = np.eye(128, dtype=bf)
    in_maps = []
    for c in range(8):
        b, g = c // 2, c % 2
        in_maps.append({
            "QT": np.ascontiguousarray(Q[b, g * SH:(g + 1) * SH, :].T).astype(f16),
            "KT": np.ascontiguousarray(K[b].T).astype(f16),
            "VT": np.ascontiguousarray(V[b].T).astype(f16),
            "WALL": wall,
            "WOA": woa, "WOB": wob, "IDN": idn,
        })
    return in_maps


LAST_RESULTS = None


def kernel(Q, K, V, WQ, WK, WV, WO, _trace=False):
    global LAST_RESULTS
    from concourse.bass_utils import run_bass_kernel_spmd

    Q = np.asarray(Q)
    K = np.asarray(K)
    V = np.asarray(V)
    nc = _get_nc()
    in_maps = make_in_maps(Q, K, V, np.asarray(WQ), np.asarray(WK),
                           np.asarray(WV), np.asarray(WO))
    res = run_bass_kernel_spmd(nc, in_maps, list(range(8)), trace=_trace)
    LAST_RESULTS = res
    out = np.empty((B, S, D), np.float32)
    for b in range(B):
        out[b, 0:SH] = res.results[2 * b]["Y"]
        out[b, SH:S] = res.results[2 * b + 1]["Y"]
    return out
